# revision 7
# baseline (speedup 1.0000x reference)
"""Localized embedding layer (separable 5x5 Gaussian stencil) on 8 trn2 cores.

Math: out[i,j,:] = sum_{|di|<=2,|dj|<=2} w(di)w(dj) H[i+di,j+dj,:] / den(i,j)
with w(d) = exp(-c*d^2), c = TILE^2/(2 sigma^2), den(i,j) = r(i)*r(j) rank-1.

Per core (32 output grid rows + 2-row halo each side, zero padded), inputs
staged host-side as bf16 (tolerance 2e-2):
  - row tiles are FLAT [128, 1024] bf16 (3D APs defeat the DVE 2x packed
    mode); i-conv pair sums t1=a1+a3, t2=a0+a4 BOTH on DVE (GpSimd shares
    DVE's SBUF ports - concurrent GpSimd tensor ops halve DVE throughput)
  - i-conv combine + j-conv on TensorE: per half, THREE bf16 matmuls
    accumulated in PSUM: B@a2 + (w1*B)@t1 + (w2*B)@t2, with B the UNIFORM
    symmetric band w(|dj|)/W_full (same weights both halves). The 8 output
    columns j in {0,1,126..129,254,255} where the uniform band is wrong
    (half-boundary crossing / grid edge) are recomputed by a strip pass
    that is ALSO pure matmuls: psf[i,jo] accumulates (w(jo-jin)/r(jo) *
    Wstrip) @ xs[:,jin,:] over the <=5 j-taps, where Wstrip is the [36,32]
    i-conv matrix (contraction dim = grid row).
  - ScalarE: one PSUM->SBUF f32 copy per row with per-row scale W_full/r(i)
  - DMA: loads on the gpsimd SWDGE ring; stores alternate between the sync
    and scalar HWDGE rings (per-ring transfers are FIFO; rings run in
    parallel)
"""

import sys
import numpy as np
import ml_dtypes

if "/opt/trn_rl_repo" not in sys.path:
    sys.path.insert(0, "/opt/trn_rl_repo")

BF16 = ml_dtypes.bfloat16

G = 256          # grid side
D = 512          # feature dim
P = 2            # grid_step halo
NC = 8           # cores
RPC = G // NC    # rows per core = 32
TILE = 448.0
SIGMA = 200.0

# strip columns: grid edges + half-boundary neighborhood
JO_LIST = [0, 1, 126, 127, 128, 129, 254, 255]
# xs slot layout: jin columns the strip needs
XS_COLS = list(range(0, 4)) + list(range(124, 132)) + list(range(252, 256))
XS_SLOT = {j: s for s, j in enumerate(XS_COLS)}

_cache = {}


def _weights():
    c = TILE * TILE / (2.0 * SIGMA * SIGMA)
    return np.exp(-c * np.arange(-P, P + 1) ** 2)   # [w2,w1,1,w1,w2] f64


def _r_vec():
    """r(i) = sum of valid 1D taps at row i (same for columns)."""
    w = _weights()
    r = np.zeros(G)
    for d in range(-P, P + 1):
        lo, hi = max(0, -d), min(G, G - d)
        r[lo:hi] += w[d + P]
    return r


def _strip_taps():
    """[(jo, [(xs_slot, w_scale), ...]), ...] for the 8 strip columns."""
    w = _weights()
    r = _r_vec()
    out = []
    for jo in JO_LIST:
        taps = []
        for dj in range(-P, P + 1):
            jin = jo + dj
            if 0 <= jin < G:
                taps.append((XS_SLOT[jin], w[dj + P] / r[jo]))
        out.append((jo, taps))
    return out


def _host_consts():
    w = _weights()
    r = _r_vec()
    w_full = w.sum()
    w1, w2 = w[1], w[0]
    # uniform symmetric band B[jout, jin] = w(|jout-jin|)/W_full, 128x128
    Bu = np.zeros((128, 128))
    for d in range(-P, P + 1):
        for jout in range(128):
            jin = jout + d
            if 0 <= jin < 128:
                Bu[jout, jin] = w[d + P] / w_full
    wmat = np.zeros((128, 3, 128), dtype=BF16)
    wmat[:, 0, :] = Bu.T.astype(BF16)
    wmat[:, 1, :] = (w1 * Bu.T).astype(BF16)
    wmat[:, 2, :] = (w2 * Bu.T).astype(BF16)
    # strip i-conv lhsT [36, 32]: Tstrip[i] = sum_k w[k] * XS[i+k]
    wstrip = np.zeros((RPC + 2 * P, RPC))
    for i in range(RPC):
        for k in range(5):
            wstrip[i + k, i] = w[k]
    # scaled copies, one per (jo, tap) pair
    taps = _strip_taps()
    ntap = sum(len(t) for _, t in taps)
    ws = np.zeros((RPC + 2 * P, ntap, RPC), dtype=BF16)
    idx = 0
    tap_index = []     # per jo: [(slot, ws_idx), ...]
    for jo, tl in taps:
        entry = []
        for slot, scale in tl:
            ws[:, idx, :] = (scale * wstrip).astype(BF16)
            entry.append((slot, idx))
            idx += 1
        tap_index.append(entry)
    # per-core scales: 1/r_row(i) (the 1/w_full j-normalizer is in Bu)
    scales, sfixes = [], []
    for c in range(NC):
        s = (1.0 / r[RPC * c: RPC * (c + 1)]).astype(np.float32)
        scales.append(np.broadcast_to(s[None, :], (128, RPC)).copy())
        # strip scale: 1 / r_row(i) (j-normalizer folded into ws)
        sf = np.zeros((128, 1), dtype=np.float32)
        sf[:RPC, 0] = (1.0 / r[RPC * c: RPC * (c + 1)]).astype(np.float32)
        sfixes.append(sf)
    return wmat, ws, tap_index, scales, sfixes


def _build_nc():
    import concourse.bass as bass
    import concourse.mybir as mybir
    import concourse.tile as tile
    from concourse import bacc

    f32 = mybir.dt.float32
    bf16 = mybir.dt.bfloat16
    add = mybir.AluOpType.add

    NR = RPC + 2 * P
    _, _, tap_index, _, _ = _get_consts()
    ntap = sum(len(t) for t in tap_index)

    nc = bacc.Bacc(None, target_bir_lowering=False, debug=False)
    x_dram = nc.declare_dram_parameter("x", [NR, G, D], bf16, isOutput=False)
    wm_dram = nc.declare_dram_parameter("wmat", [128, 3, 128], bf16, isOutput=False)
    wf_dram = nc.declare_dram_parameter("wstrip", [NR, ntap, RPC], bf16, isOutput=False)
    sc_dram = nc.declare_dram_parameter("scale", [128, RPC], f32, isOutput=False)
    sf_dram = nc.declare_dram_parameter("sfix", [128, 1], f32, isOutput=False)
    y_dram = nc.declare_dram_parameter("y", [RPC, G, D], f32, isOutput=True)

    NXS = len(XS_COLS)

    with tile.TileContext(nc) as tc:
        with (
            tc.tile_pool(name="const", bufs=1) as cpool,
            tc.tile_pool(name="x", bufs=NR) as xpool,
            tc.tile_pool(name="tmp", bufs=3) as tpool,
            tc.tile_pool(name="out", bufs=5) as opool,
            tc.tile_pool(name="fix", bufs=1) as fpool,
            tc.tile_pool(name="psum", bufs=3, space="PSUM") as ppool,
            tc.tile_pool(name="psfix", bufs=1, space="PSUM") as pfpool,
        ):
            wt = cpool.tile([128, 3, 128], bf16)
            nc.gpsimd.dma_start(wt[:], wm_dram[:])
            wft = cpool.tile([NR, ntap, RPC], bf16)
            nc.gpsimd.dma_start(wft[:], wf_dram[:])
            st = cpool.tile([128, RPC], f32)
            nc.gpsimd.dma_start(st[:], sc_dram[:])
            sft = cpool.tile([128, 1], f32)
            nc.gpsimd.dma_start(sft[:], sf_dram[:])

            # prefetch ALL input rows up front (gpsimd SWDGE ring)
            xt = {}
            for r in range(NR):
                t = xpool.tile([128, 1024], bf16, tag="xrow")
                nc.gpsimd.dma_start(
                    t[:].rearrange("p (h d) -> p h d", h=2),
                    x_dram[r].rearrange("(h p) d -> p h d", p=128),
                )
                xt[r] = t

            # strip input: xs[r, slot, d] = x[r, XS_COLS[slot], d]
            xs = fpool.tile([NR, NXS, D], bf16, tag="xs")
            nc.gpsimd.dma_start(xs[:, 0:4, :], x_dram[:, 0:4, :])
            nc.gpsimd.dma_start(xs[:, 4:12, :], x_dram[:, 124:132, :])
            nc.gpsimd.dma_start(xs[:, 12:16, :], x_dram[:, 252:256, :])

            # strip pass p handles JO_LIST[2p], JO_LIST[2p+1]
            def emit_strip_pass(p):
                psf = pfpool.tile([RPC, 2, D], f32, tag="psf")
                for c in range(2):
                    entry = tap_index[2 * p + c]
                    for k, (slot, widx) in enumerate(entry):
                        nc.tensor.matmul(
                            psf[:, c, :], wft[:, widx, :], xs[:, slot, :],
                            start=(k == 0), stop=(k == len(entry) - 1),
                        )
                fs = fpool.tile([RPC, 2, D], f32, tag=f"fs{p}")
                nc.scalar.mul(fs[:], psf[:], sft[0:RPC, 0:1])
                jo0 = JO_LIST[2 * p]
                nc.sync.dma_start(y_dram[:, jo0:jo0 + 2, :], fs[:])

            # ---- main loop (strip passes interleaved into first rows) ----
            for i in range(RPC):
                a0, a1, a2, a3, a4 = (xt[i + k] for k in range(5))
                t1 = tpool.tile([128, 1024], bf16, tag="t1")
                nc.vector.tensor_tensor(t1[:], a1[:], a3[:], add)
                t2 = tpool.tile([128, 1024], bf16, tag="t2")
                nc.vector.tensor_tensor(t2[:], a0[:], a4[:], add)
                ps = ppool.tile([128, 2, D], f32, tag="ps")
                for hm in range(2):
                    sl = slice(512 * hm, 512 * hm + 512)
                    nc.tensor.matmul(
                        ps[:, hm, :], wt[:, 0, :], a2[:, sl],
                        start=True, stop=False,
                    )
                    nc.tensor.matmul(
                        ps[:, hm, :], wt[:, 1, :], t1[:, sl],
                        start=False, stop=False,
                    )
                    nc.tensor.matmul(
                        ps[:, hm, :], wt[:, 2, :], t2[:, sl],
                        start=False, stop=True,
                    )
                ob = opool.tile([128, 2, D], f32, tag="ob")
                nc.scalar.mul(ob[:], ps[:], st[:, i:i + 1])
                ring = nc.sync if i % 2 == 0 else nc.scalar
                ring.dma_start(y_dram[i, 2:126, :], ob[2:126, 0, :])
                ring.dma_start(y_dram[i, 130:254, :], ob[2:126, 1, :])
                if i < 4:
                    emit_strip_pass(i)
    nc.finalize()
    return nc


def _get_consts():
    if "consts" not in _cache:
        _cache["consts"] = _host_consts()
    return _cache["consts"]


def _get_program():
    if "nc" not in _cache:
        _cache["nc"] = _build_nc()
    return _cache["nc"], _get_consts()


def _in_maps(H):
    wmat, ws, tap_index, scales, sfixes = _get_consts()
    H3 = H.reshape(G, G, D)
    Hp = np.zeros((G + 2 * P, G, D), dtype=BF16)
    Hp[P:P + G] = H3.astype(BF16)
    in_maps = []
    for c in range(NC):
        shard = np.ascontiguousarray(Hp[RPC * c: RPC * c + RPC + 2 * P])
        in_maps.append(
            {"x": shard, "wmat": wmat, "wstrip": ws,
             "scale": scales[c], "sfix": sfixes[c]}
        )
    return in_maps


def _unshard(res):
    return np.concatenate(
        [res[c]["y"].reshape(RPC * G, D) for c in range(NC)], axis=0
    )


def kernel(H, xy=None):
    from concourse.bass_utils import run_bass_kernel_spmd

    nc, _ = _get_program()
    res = run_bass_kernel_spmd(nc, _in_maps(H), list(range(NC))).results
    return _unshard(res)
